# revision 3
# baseline (speedup 1.0000x reference)
"""Trainium2 Bass kernel v5 for nn_AttentionLayer (B=2, S=2048, HID=1024, 16 heads x 64).

Sharding: 8 cores = 2 batches x 4 head-groups (4 heads each); no collectives.

v5 design (ScalarE exp is the wall; everything else hides under it):
  - 8 rounds: (jt in 0..1) x (fq in 0..3), f-width 512 per round.
  - per round, 16 t-tiles; each writes one [128, 1024] PSUM staging tile
    (h0 in cols 0:512, h1 in 512:1024; N=512 matmuls, dd row-tiled).
  - ONE exp ACTIVATE of FD=1024 per tt (measured steady pitch 1376 ns;
    double-buffered staging keeps ScalarE gap-free) -> E [128, 1024] bf16.
  - PV: M=65 stationary (V plus ones column => softmax denominator rides the
    PV matmul); accumulate [65, 512] per head across all 16 tt of the round.
  - PSUM: 2x staging (2 banks) + 2x cacc (1 bank) + 2x proj acc (1 bank) = 8.
  - projections (Q/K/V blocks) issued as PE fillers inside the exp stream;
    head only computes k(0,0)+q(0,0); ACT table preload + PE HAM warmup at t=0.
  - input DMA: one 3-D DMA per 512-col block per tensor, critical prefix first.
"""
import numpy as np

B, S, HID = 2, 2048, 1024
NUM_HEADS, HEAD_DIM = 16, 64
G = 4                 # head-groups (cores per batch)
HPC = 4               # heads per core
JW = HPC * HEAD_DIM   # 256 W columns per core
NCC = HID // 128      # 8 contraction chunks
NT = S // 128         # 16 t tiles
NBLK = 4              # 512-col blocks per tensor
SCALE = 1.0 / np.sqrt(float(HEAD_DIM))

_cached = None


def _build():
    import contextlib
    import concourse.bass as bass
    import concourse.tile as tile
    from concourse import bacc, mybir

    F32 = mybir.dt.float32
    BF16 = mybir.dt.bfloat16
    Act = mybir.ActivationFunctionType

    nc = bacc.Bacc("TRN2", target_bir_lowering=False, debug=False, num_devices=8)

    fromT = nc.dram_tensor("fromT", (HID, S), BF16, kind="ExternalInput").ap()
    toT = nc.dram_tensor("toT", (HID, S), BF16, kind="ExternalInput").ap()
    wq = nc.dram_tensor("wq", (HID, JW), BF16, kind="ExternalInput").ap()
    wk = nc.dram_tensor("wk", (HID, JW), BF16, kind="ExternalInput").ap()
    wv = nc.dram_tensor("wv", (HID, JW), BF16, kind="ExternalInput").ap()
    bq = nc.dram_tensor("bq", (JW, 1), F32, kind="ExternalInput").ap()
    bk = nc.dram_tensor("bk", (JW, 1), F32, kind="ExternalInput").ap()
    bv = nc.dram_tensor("bv", (1, JW), F32, kind="ExternalInput").ap()
    out = nc.dram_tensor("out", (JW, S), F32, kind="ExternalOutput").ap()

    with tile.TileContext(nc) as tc:
        with contextlib.ExitStack() as es:
            persist = es.enter_context(tc.tile_pool(name="persist", bufs=1))
            pstage = es.enter_context(tc.tile_pool(name="pstage", bufs=1, space="PSUM"))
            pacc = es.enter_context(tc.tile_pool(name="pacc", bufs=1, space="PSUM"))
            pproj = es.enter_context(tc.tile_pool(name="pproj", bufs=1, space="PSUM"))
            work = es.enter_context(tc.tile_pool(name="work", bufs=1))

            # ---- DMA issue order: critical prefix first, split into c-halves
            # so k(0,0)/q(0,0) chunk matmuls can start on the first half.
            xt = {}

            def load_half(nm, src, b, h):
                t = work.tile([128, NCC // 2, 512], BF16, tag="xt", bufs=16,
                              name=f"x_{nm}{b}{h}")
                xt[(nm, b, h)] = t
                nc.sync.dma_start(
                    t[:],
                    src.rearrange("(c p) s -> p c s", p=128)[
                        :, 4 * h:4 * h + 4, 512 * b:512 * b + 512])

            def load_w_half(dst, src, h):
                nc.sync.dma_start(
                    dst[:, 4 * h:4 * h + 4, :],
                    src.rearrange("(c p) j -> p c j", p=128)[:, 4 * h:4 * h + 4, :])

            wk_sb = persist.tile([128, NCC, JW], BF16, tag="wk")
            wq_sb = persist.tile([128, NCC, JW], BF16, tag="wq")
            wv_sb = persist.tile([128, NCC, JW], BF16, tag="wv")
            load_w_half(wk_sb, wk, 0)
            load_half("t", toT, 0, 0)
            load_w_half(wq_sb, wq, 0)
            load_half("f", fromT, 0, 0)
            load_w_half(wk_sb, wk, 1)
            load_half("t", toT, 0, 1)
            load_w_half(wq_sb, wq, 1)
            load_half("f", fromT, 0, 1)
            b_sb = {}
            for nm, src in (("bq", bq), ("bk", bk)):
                for jt in range(2):
                    t = persist.tile([128, 1], F32, tag=f"{nm}{jt}")
                    nc.sync.dma_start(t[:], src[128 * jt:128 * jt + 128, 0:1])
                    b_sb[(nm, jt)] = t
            load_w_half(wv_sb, wv, 0)
            load_w_half(wv_sb, wv, 1)
            bv_row = persist.tile([1, JW], F32, tag="bvrow")
            nc.sync.dma_start(bv_row[:], bv[0:1, :])
            for b in (1, 2, 3):
                load_half("t", toT, b, 0)
                load_half("t", toT, b, 1)
                load_half("f", fromT, b, 0)
                load_half("f", fromT, b, 1)

            # ---- t~0 machinery on non-sync engines
            # ACT table preload (overlaps input DMA)
            tiny_i = persist.tile([1, 16], F32, tag="tinyi")
            nc.vector.memset(tiny_i[:], 0.0)
            tiny_o = persist.tile([1, 16], BF16, tag="tinyo")
            nc.scalar.activation(tiny_o[:], tiny_i[:], Act.Exp, bias=0.0, scale=1.0)
            # PE HAM warmup: ~10 dummy matmuls on zeros (~4.3us cold)
            warm_w = persist.tile([128, 128], BF16, tag="warmw")
            nc.vector.memset(warm_w[:], 0.0)
            warm_x = persist.tile([128, 512], BF16, tag="warmx")
            nc.vector.memset(warm_x[:], 0.0)
            for i in range(16):
                pw = pproj.tile([128, 512], F32, tag="proj", bufs=2, name=f"warm{i}")
                nc.tensor.matmul(pw[:], warm_w[:], warm_x[:], start=True, stop=True)
            # bv broadcast to all partitions (one-time)
            bv_bc = persist.tile([128, JW], F32, tag="bvbc")
            nc.gpsimd.partition_broadcast(bv_bc[:], bv_row[:])

            # ---- persistent projection outputs (bf16)
            kt = [persist.tile([128, S], BF16, tag=f"kt{jt}", name=f"kt{jt}") for jt in range(2)]
            qt = [persist.tile([128, S], BF16, tag=f"qt{jt}", name=f"qt{jt}") for jt in range(2)]
            vp = [persist.tile([128, HPC, 65], BF16, tag=f"vp{tt}", name=f"vp{tt}") for tt in range(NT)]

            # ---- projection helpers
            def k_proj(jt, tb):
                acc = pproj.tile([128, 512], F32, tag="proj", bufs=2,
                                 name=f"kacc{jt}_{tb}")
                for c in range(NCC):
                    x = xt[("t", tb, c // 4)]
                    nc.tensor.matmul(acc[:], wk_sb[:, c, 128 * jt:128 * jt + 128],
                                     x[:, c % 4, :], start=(c == 0), stop=(c == NCC - 1))
                nc.vector.tensor_scalar_add(kt[jt][:, 512 * tb:512 * tb + 512],
                                            acc[:], b_sb[("bk", jt)][:])

            def q_proj(jt, fq):
                acc = pproj.tile([128, 512], F32, tag="proj", bufs=2,
                                 name=f"qacc{jt}_{fq}")
                for c in range(NCC):
                    x = xt[("f", fq, c // 4)]
                    nc.tensor.matmul(acc[:], wq_sb[:, c, 128 * jt:128 * jt + 128],
                                     x[:, c % 4, :], start=(c == 0), stop=(c == NCC - 1))
                nc.vector.tensor_scalar_add(qt[jt][:, 512 * fq:512 * fq + 512],
                                            acc[:], b_sb[("bq", jt)][:])

            def v_proj(tt):
                b, t2 = divmod(tt, 4)
                accv = pproj.tile([128, 512], F32, tag="proj", bufs=2,
                                  name=f"vacc{tt}")
                for c in range(NCC):
                    x = xt[("t", b, c // 4)]
                    nc.tensor.matmul(accv[:, 0:JW],
                                     x[:, c % 4, 128 * t2:128 * t2 + 128],
                                     wv_sb[:, c, :], start=(c == 0), stop=(c == NCC - 1))
                nc.vector.memset(vp[tt][:, :, 64], 1.0)
                nc.vector.tensor_add(
                    vp[tt][:, :, 0:64],
                    accv[:, 0:JW].rearrange("p (k e) -> p k e", k=HPC),
                    bv_bc[:, 0:JW].rearrange("p (k e) -> p k e", k=HPC))

            # ---- head: minimal prefix + early V tiles (fill DMA-wait gaps)
            k_proj(0, 0)
            q_proj(0, 0)
            for _tt in range(4):
                v_proj(_tt)

            # ---- filler schedule: {round: {tt-slot: [thunks]}}
            filler = {r: {t: [] for t in range(NT)} for r in range(8)}
            filler[0][1] += [lambda: k_proj(0, 1)]
            filler[0][3] += [lambda: v_proj(4)]
            filler[0][4] += [lambda: v_proj(5)]
            filler[0][5] += [lambda: k_proj(0, 2)]
            filler[0][6] += [lambda: v_proj(6), lambda: v_proj(7)]
            filler[0][7] += [lambda: v_proj(8)]
            filler[0][8] += [lambda: v_proj(9)]
            filler[0][9] += [lambda: k_proj(0, 3)]
            filler[0][10] += [lambda: v_proj(10), lambda: v_proj(11)]
            filler[0][11] += [lambda: v_proj(12)]
            filler[0][12] += [lambda: v_proj(13)]
            filler[0][13] += [lambda: q_proj(0, 1)]
            filler[0][14] += [lambda: v_proj(14), lambda: v_proj(15)]
            filler[1][0] += [lambda: q_proj(0, 2)]
            filler[1][4] += [lambda: k_proj(1, 0)]
            filler[1][10] += [lambda: k_proj(1, 1)]
            filler[2][0] += [lambda: q_proj(0, 3)]
            filler[2][4] += [lambda: k_proj(1, 2)]
            filler[2][10] += [lambda: k_proj(1, 3)]
            filler[3][0] += [lambda: q_proj(1, 0)]
            filler[3][8] += [lambda: q_proj(1, 1)]
            filler[4][0] += [lambda: q_proj(1, 2)]
            filler[5][0] += [lambda: q_proj(1, 3)]

            # ---- rounds
            rounds = [(0, 0), (0, 1), (0, 2), (0, 3), (1, 0), (1, 1), (1, 2), (1, 3)]
            for r, (jt, fq) in enumerate(rounds):
                fo = 512 * fq
                cacc = [pacc.tile([65, 512], F32, tag="cacc", bufs=2, name=f"cacc{r}_{dd}")
                        for dd in range(2)]
                E = {}

                def scores_exp(tt):
                    sp = pstage.tile([128, 1024], F32, tag="sp", bufs=2,
                                     name=f"sp{r}_{tt}")
                    for dd in range(2):
                        nc.tensor.matmul(
                            sp[:, 512 * dd:512 * dd + 512],
                            kt[jt][64 * dd:64 * dd + 64, 128 * tt:128 * tt + 128],
                            qt[jt][64 * dd:64 * dd + 64, fo:fo + 512],
                            start=True, stop=True, tile_position=(64 * dd, 0))
                    e = work.tile([128, 1024], BF16, tag="et", bufs=4,
                                  name=f"e{r}_{tt}")
                    nc.scalar.activation(e[:], sp[:], Act.Exp, bias=0.0, scale=SCALE)
                    E[tt] = e

                def pv(tt):
                    for dd in range(2):
                        k_local = 2 * jt + dd
                        nc.tensor.matmul(
                            cacc[dd][:],
                            vp[tt][:, k_local, :],
                            E[tt][:, 512 * dd:512 * dd + 512],
                            start=(tt == 0), stop=(tt == NT - 1))

                for tt in range(NT):
                    scores_exp(tt)
                    for th in filler[r][tt]:
                        th()
                    if tt > 0:
                        pv(tt - 1)
                pv(NT - 1)

                # epilogue: normalize by the denominator row and store; copy
                # cacc out first so the PSUM bank frees for the next round.
                for dd in range(2):
                    k_local = 2 * jt + dd
                    sbf = work.tile([65, 512], F32, tag="sbf", bufs=2,
                                    name=f"sbf{r}_{dd}")
                    nc.vector.tensor_copy(sbf[:], cacc[dd][:])
                    # reciprocal_approx_fast / partition_broadcast misread at a
                    # nonzero partition offset on HW - DMA den to partition 0.
                    den0 = work.tile([1, 512], F32, tag="den0", bufs=2,
                                     name=f"den0{r}_{dd}")
                    nc.sync.dma_start(den0[:], sbf[64:65, :])
                    rcp = work.tile([1, 512], F32, tag="rcp", bufs=2,
                                    name=f"rcp{r}_{dd}")
                    nc.vector.reciprocal_approx_fast(rcp[:], den0[:])
                    rcpb = work.tile([64, 512], F32, tag="rcpb", bufs=2,
                                     name=f"rcpb{r}_{dd}")
                    nc.gpsimd.partition_broadcast(rcpb[:], rcp[:])
                    so = work.tile([64, 512], F32, tag="so", bufs=2,
                                   name=f"so{r}_{dd}")
                    nc.vector.tensor_mul(so[:], sbf[0:64, :], rcpb[:])
                    nc.sync.dma_start(
                        out[64 * k_local:64 * k_local + 64, fo:fo + 512], so[:])

    nc.compile()
    return nc


def _get_nc():
    global _cached
    if _cached is None:
        _cached = _build()
    return _cached


def _numpy_fallback(from_tensor, to_tensor, attention_mask, Wq, bq, Wk, bk, Wv, bv):
    b, f, _ = from_tensor.shape
    t = to_tensor.shape[1]
    h, d = NUM_HEADS, HEAD_DIM
    q = (from_tensor @ Wq + bq).reshape(b, f, h, d).transpose(0, 2, 1, 3)
    k = (to_tensor @ Wk + bk).reshape(b, t, h, d).transpose(0, 2, 1, 3)
    v = (to_tensor @ Wv + bv).reshape(b, t, h, d).transpose(0, 2, 1, 3)
    scores = np.einsum("bhfd,bhtd->bhft", q, k) * (1.0 / np.sqrt(float(d)))
    adder = (1.0 - attention_mask[:, None].astype(np.float32)) * -10000.0
    scores = scores + adder
    scores = scores - scores.max(axis=-1, keepdims=True)
    e = np.exp(scores)
    probs = e / e.sum(axis=-1, keepdims=True)
    ctx = np.einsum("bhft,bhtd->bhfd", probs, v)
    return ctx.transpose(0, 2, 1, 3).reshape(b, f, h * d).astype(np.float32)


def _make_in_maps(from_tensor, to_tensor, Wq, bq, Wk, bk, Wv, bv):
    import ml_dtypes
    bf16 = ml_dtypes.bfloat16
    fromT = [np.ascontiguousarray(from_tensor[b].T).astype(bf16) for b in range(B)]
    toT = [np.ascontiguousarray(to_tensor[b].T).astype(bf16) for b in range(B)]
    in_maps = []
    for core in range(8):
        b, g = divmod(core, G)
        j0 = JW * g
        in_maps.append({
            "fromT": fromT[b],
            "toT": toT[b],
            "wq": np.ascontiguousarray(Wq[:, j0:j0 + JW]).astype(bf16),
            "wk": np.ascontiguousarray(Wk[:, j0:j0 + JW]).astype(bf16),
            "wv": np.ascontiguousarray(Wv[:, j0:j0 + JW]).astype(bf16),
            "bq": np.ascontiguousarray(bq[j0:j0 + JW].reshape(JW, 1)),
            "bk": np.ascontiguousarray(bk[j0:j0 + JW].reshape(JW, 1)),
            "bv": np.ascontiguousarray(bv[j0:j0 + JW].reshape(1, JW)),
        })
    return in_maps


def profile_exec_time(inputs):
    """Rerun on HW with NTFF tracing; returns whole-NEFF exec time in ns."""
    from concourse import bass_utils
    nc = _get_nc()
    in_maps = _make_in_maps(
        np.asarray(inputs["from_tensor"], dtype=np.float32),
        np.asarray(inputs["to_tensor"], dtype=np.float32),
        np.asarray(inputs["Wq"], dtype=np.float32),
        np.asarray(inputs["bq"], dtype=np.float32),
        np.asarray(inputs["Wk"], dtype=np.float32),
        np.asarray(inputs["bk"], dtype=np.float32),
        np.asarray(inputs["Wv"], dtype=np.float32),
        np.asarray(inputs["bv"], dtype=np.float32))
    res = bass_utils.run_bass_kernel_spmd(nc, in_maps, core_ids=list(range(8)),
                                          trace=True)
    profile_exec_time.last_results = res
    return res.exec_time_ns


def kernel(**inputs) -> np.ndarray:
    from_tensor = np.asarray(inputs["from_tensor"], dtype=np.float32)
    to_tensor = np.asarray(inputs["to_tensor"], dtype=np.float32)
    attention_mask = np.asarray(inputs["attention_mask"])
    Wq = np.asarray(inputs["Wq"], dtype=np.float32)
    bq = np.asarray(inputs["bq"], dtype=np.float32)
    Wk = np.asarray(inputs["Wk"], dtype=np.float32)
    bk = np.asarray(inputs["bk"], dtype=np.float32)
    Wv = np.asarray(inputs["Wv"], dtype=np.float32)
    bv = np.asarray(inputs["bv"], dtype=np.float32)

    if not np.all(attention_mask == 1):
        return _numpy_fallback(from_tensor, to_tensor, attention_mask,
                               Wq, bq, Wk, bk, Wv, bv)

    from concourse import bass_utils

    nc = _get_nc()
    in_maps = _make_in_maps(from_tensor, to_tensor, Wq, bq, Wk, bk, Wv, bv)
    res = bass_utils.run_bass_kernel_spmd(nc, in_maps, core_ids=list(range(8)))
    kernel.last_results = res

    output = np.empty((B, S, HID), dtype=np.float32)
    for core in range(8):
        b, g = divmod(core, G)
        j0 = JW * g
        output[b, :, j0:j0 + JW] = res.results[core]["out"].T
    return output


if __name__ == "__main__":
    rng = np.random.default_rng(0)
    ins = {
        "from_tensor": rng.standard_normal((B, S, HID), dtype=np.float32),
        "to_tensor": rng.standard_normal((B, S, HID), dtype=np.float32),
        "attention_mask": np.ones((B, S, S), dtype=np.int32),
        "Wq": rng.standard_normal((HID, HID), dtype=np.float32) * 0.02,
        "bq": rng.standard_normal((HID,), dtype=np.float32) * 0.01,
        "Wk": rng.standard_normal((HID, HID), dtype=np.float32) * 0.02,
        "bk": rng.standard_normal((HID,), dtype=np.float32) * 0.01,
        "Wv": rng.standard_normal((HID, HID), dtype=np.float32) * 0.02,
        "bv": rng.standard_normal((HID,), dtype=np.float32) * 0.01,
    }
    got = kernel(**ins)
    want = _numpy_fallback(**ins)
    err = np.abs(got - want).max() / np.abs(want).max()
    print("self-test rel err:", err)
